# revision 5
# baseline (speedup 1.0000x reference)
"""3D Haar DWT (single level) on 8 Trainium2 NeuronCores — bf16 I/O.

Input:  data (2, 8, 128, 128, 128) f32 + six banded Haar matrices.
Output: tuple of 8 subbands (LLL, LLH, LHL, LHH, HLL, HLH, HHL, HHH),
        each (2, 8, 64, 64, 64) f32.  Band letters are [D][H][W] filters.

The kernel is HBM-bandwidth bound, so all device I/O is bf16 (host casts
f32<->bf16; rel-err ~3e-3, well inside tolerance).  Per core (2 (n,c)
slices): 8 MiB in + 8 MiB out.

Per 16-plane chunk ([128 h][16 d][128 w], w pre-deinterleaved on host so
even w' cols 0-63, odd cols 64-127):
  - D-butterfly on DVE: dsum/ddiff = d_even +/- d_odd, all step-1 bf16
    tensor_tensor ops (2x DVE mode).
  - H-stage + W-butterfly on PE: psum_q = AH@Xe +/- AH@Xo via PSUM
    accumulation with +AH / -AH weight sets (8 bf16 matmuls, N=512).
    AH rows 0-63 = H-low, 64-127 = H-high, pre-scaled by v_w*v_d.
  - PSUM (f32) -> SBUF acc (bf16) cast copies spread over ScalarE, DVE,
    GpSimd.
  - Output: 4 quadrant acc tiles, DMA'd per 2 chunks (256 KiB each).
"""

import sys

for _p in ("/opt/trn_rl_repo", "/root/.axon_site/_ro/trn_rl_repo"):
    if _p not in sys.path:
        sys.path.append(_p)

import json

import numpy as np
import ml_dtypes

import concourse.bass as bass
import concourse.tile as tile
import concourse.mybir as mybir
from concourse.bass_utils import run_bass_kernel_spmd

N_CORES = 8
D = H = W = 128
SLICES_PER_CORE = 2
PLANES_PER_CHUNK = 16
CHUNKS_PER_SLICE = D // PLANES_PER_CHUNK   # 8
F32 = mybir.dt.float32
BF16 = mybir.dt.bfloat16
NPBF16 = ml_dtypes.bfloat16


# The pinned walrus build rejects instructions carrying more than one
# sync-wait ("Too many sync wait commands", CoreV3GenImpl setupSyncWait).
# Tile's wait assignment freely attaches several.  Post-process the
# serialized BIR: move all-but-one wait of any instruction onto fresh
# single-wait NoOps inserted just before it on the same engine (same
# per-engine program order -> identical semantics).
_orig_to_json_bytes = bass.Bass.to_json_bytes


def _split_multi_waits(data: bytes) -> bytes:
    d = json.loads(data)
    ctr = 0
    changed = False
    for f in d.get("functions", []):
        for blk in f.get("blocks", []):
            insts = blk.get("instructions", [])
            out = []
            for inst in insts:
                si = inst.get("sync_info") or {}
                ow = si.get("on_wait") or []
                if len(ow) > 1:
                    changed = True
                    for w in ow[:-1]:
                        ctr += 1
                        out.append(
                            {
                                "name": f"WS-{ctr}",
                                "opcode": "NoOp",
                                "engine": inst.get("engine"),
                                "ins": [],
                                "outs": [],
                                "debug": inst.get("debug"),
                                "sync_info": {
                                    "on_update": [],
                                    "on_wait": [w],
                                },
                            }
                        )
                    si["on_wait"] = [ow[-1]]
                out.append(inst)
            blk["instructions"] = out
    if not changed:
        return data
    return json.dumps(d).encode()


def _to_json_bytes_split(self):
    return _split_multi_waits(_orig_to_json_bytes(self))


bass.Bass.to_json_bytes = _to_json_bytes_split


def build_bass():
    """Build the per-core SPMD Bass program (bf16 I/O)."""
    nc = bass.Bass("TRN2", target_bir_lowering=False, debug=False)

    # x: [slice][h][d][w-deint] bf16; per-partition input DMA lines are
    # 4 KiB contiguous (16 d-planes x 256 B).
    x = nc.dram_tensor("x", (SLICES_PER_CORE, H, D, W), BF16, kind="ExternalInput")
    # w2: cols 0-127 = AH^T (scaled), cols 128-255 = -AH^T.
    w2 = nc.dram_tensor("w2", (H, 256), BF16, kind="ExternalInput")
    # y: [slice][quad 2*d_hi+w_hi][p' band][e*64+w'] bf16
    y = nc.dram_tensor(
        "y", (SLICES_PER_CORE, 4, 128, D // 2 * (W // 2)), BF16,
        kind="ExternalOutput",
    )

    PREFETCH = 6

    with tile.TileContext(nc) as tc:
        with (
            tc.tile_pool(name="consts", bufs=1) as cpool,
            tc.tile_pool(name="inp", bufs=4) as ipool,
            tc.tile_pool(name="mid", bufs=3) as mpool,
            tc.tile_pool(name="psum", bufs=2, space="PSUM") as ppool,
            tc.tile_pool(name="acc", bufs=2) as apool,
        ):
            w2_t = cpool.tile([H, 256], BF16, tag="w2")
            nc.scalar.dma_start(w2_t[:], w2.ap())
            wp = w2_t[:, 0:128]
            wn = w2_t[:, 128:256]

            jobs = [(s, c) for s in range(SLICES_PER_CORE)
                    for c in range(CHUNKS_PER_SLICE)]
            NJ = len(jobs)
            tiles = {}
            mids = {}
            psums = {}
            accs = {}

            def issue_in(j):
                s, c = jobs[j]
                t = ipool.tile([H, PLANES_PER_CHUNK * W], BF16,
                               tag="chunk", name="chunk", bufs=PREFETCH + 2)
                nc.scalar.dma_start(
                    t[:].rearrange("h (d w) -> h d w", w=W),
                    x.ap()[s][:, c * PLANES_PER_CHUNK:(c + 1) * PLANES_PER_CHUNK, :],
                )
                tiles[j] = t

            def stage_tt(j):
                # D-butterfly (bf16, step-1 inner dim -> DVE 2x mode).
                # dsum/ddiff layout [h][par][e][w'] so matmul rhs slices
                # (Se, So, Te, To) are fully contiguous 512 columns.
                chunk = tiles.pop(j)
                planes = chunk[:].rearrange("h (d w) -> h d w", w=W)
                d_even = planes[:, 0::2, :]   # [128, 8, 128]
                d_odd = planes[:, 1::2, :]
                dsum = mpool.tile([H, 1024], BF16, tag="dsum", name="dsum")
                ddif = mpool.tile([H, 1024], BF16, tag="ddif", name="ddif")
                ds4 = dsum[:].rearrange("h (p e w) -> h p e w", p=2, w=64)
                dd4 = ddif[:].rearrange("h (p e w) -> h p e w", p=2, w=64)
                # DVE: 2 ADDs + 1 SUB; GpSimd: 1 SUB (GpSimd cannot read
                # PSUM, so it can't help with the cast copies).
                for par in (0, 1):
                    wsl = slice(par * 64, par * 64 + 64)
                    nc.vector.tensor_add(ds4[:, par], d_even[:, :, wsl],
                                         d_odd[:, :, wsl])
                    eng = nc.gpsimd if par == 1 else nc.vector
                    eng.tensor_sub(dd4[:, par], d_even[:, :, wsl],
                                   d_odd[:, :, wsl])
                mids[j] = (dsum, ddif)

            def stage_mm(j):
                # H-matmul + W-butterfly folded into PSUM accumulation:
                #   q0 (Wlo) = AH@Se + AH@So      q1 (Whi) = AH@Se - AH@So
                #   q2 (Wlo) = AH@Te + AH@To      q3 (Whi) = AH@Te - AH@To
                dsum, ddif = mids.pop(j)
                Se = dsum[:, 0:512]
                So = dsum[:, 512:1024]
                Te = ddif[:, 0:512]
                To = ddif[:, 512:1024]
                ps = [ppool.tile([128, 512], F32, tag=f"q{q}", name=f"q{q}")
                      for q in range(4)]
                nc.tensor.matmul(ps[0][:], wp, Se, start=True, stop=False)
                nc.tensor.matmul(ps[1][:], wp, Se, start=True, stop=False)
                nc.tensor.matmul(ps[2][:], wp, Te, start=True, stop=False)
                nc.tensor.matmul(ps[3][:], wp, Te, start=True, stop=False)
                nc.tensor.matmul(ps[0][:], wp, So, start=False, stop=True)
                nc.tensor.matmul(ps[2][:], wp, To, start=False, stop=True)
                nc.tensor.matmul(ps[1][:], wn, So, start=False, stop=True)
                nc.tensor.matmul(ps[3][:], wn, To, start=False, stop=True)
                psums[j] = ps

            def stage_out(j):
                # PSUM f32 -> acc bf16 cast copies; 3 on ScalarE, 1 on DVE.
                s, c = jobs[j]
                ps = psums.pop(j)
                slot = c % 2
                if slot == 0:
                    accs[s] = {
                        q: apool.tile([128, 1024], BF16, tag=f"acc{q}",
                                      name=f"acc{q}")
                        for q in range(4)
                    }
                acc_tiles = accs[s]
                copy_eng = (nc.scalar.copy, nc.scalar.copy,
                            _vcopy(nc), nc.scalar.copy)
                for q in range(4):
                    dst = acc_tiles[q][:, slot * 512:(slot + 1) * 512]
                    copy_eng[q](dst, ps[q][:])
                if slot == 1:
                    g = c // 2   # 16-e output group within the slice
                    for q in range(4):
                        nc.sync.dma_start(
                            y.ap()[s, q][:, g * 1024:(g + 1) * 1024],
                            acc_tiles[q][:],
                        )

            # Software-pipelined emission: per-engine program order would
            # otherwise serialize DVE between chunk j's casts and chunk
            # j+1's butterflies.  Interleave stages from different chunks
            # so every engine always has an independent chunk in flight.
            for j in range(min(PREFETCH, NJ)):
                issue_in(j)
            for j in range(NJ + 2):
                if j + PREFETCH < NJ:
                    issue_in(j + PREFETCH)
                if j < NJ:
                    stage_tt(j)
                if 1 <= j < NJ + 1:
                    stage_mm(j - 1)
                if j >= 2:
                    stage_out(j - 2)

    return nc


def _vcopy(nc):
    return nc.vector.tensor_copy


def _gcopy(nc):
    return nc.gpsimd.tensor_copy


_NC_CACHE = None


def _get_nc():
    global _NC_CACHE
    if _NC_CACHE is None:
        _NC_CACHE = build_bass()
    return _NC_CACHE


def _host_prep_weights(inputs):
    l0 = np.asarray(inputs["matrix_low_0"], dtype=np.float64)   # (64,128)
    g0 = np.asarray(inputs["matrix_high_0"], dtype=np.float64)  # (64,128)
    l1 = np.asarray(inputs["matrix_low_1"], dtype=np.float64)   # (128,64)
    l2 = np.asarray(inputs["matrix_low_2"], dtype=np.float64)   # (64,128)
    v_w = l1[0, 0]
    v_d = l2[0, 0]
    ah = np.concatenate([l0, g0], axis=0)          # (128,128) rows = bands
    whT = (ah.T * (v_w * v_d))                     # (128 h, 128 band)
    w2 = np.concatenate([whT, -whT], axis=1)       # (128, 256)
    return np.ascontiguousarray(w2.astype(NPBF16))


def run(inputs, trace=False, **kwargs):
    """Run the kernel; returns (bands_tuple, BassKernelResults)."""
    data = np.asarray(inputs["data"])
    assert data.shape == (2, 8, D, H, W) and data.dtype == np.float32
    w2 = _host_prep_weights(inputs)

    # [nc][d][h][w] -> [nc][h][d][w-deinterleaved] bf16
    xf = data.reshape(16, D, H, W).transpose(0, 2, 1, 3)      # [nc][h][d][w]
    xf = xf.reshape(16, H, D, W // 2, 2).transpose(0, 1, 2, 4, 3)
    xb = np.ascontiguousarray(xf.reshape(16, H, D, W).astype(NPBF16))

    in_maps = [{"x": xb[2 * k: 2 * k + 2], "w2": w2} for k in range(N_CORES)]

    nc = _get_nc()
    res = run_bass_kernel_spmd(
        nc, in_maps, core_ids=list(range(N_CORES)), trace=trace, **kwargs
    )

    # y[k]: (2, 4, 128, 4096) bf16 -> [s][quad][p'][e][w']
    bands = [np.empty((2, 8, D // 2, H // 2, W // 2), np.float32)
             for _ in range(8)]
    for k in range(N_CORES):
        yk = np.asarray(res.results[k]["y"]).reshape(
            SLICES_PER_CORE, 4, 128, D // 2, W // 2
        ).astype(np.float32)
        for s in range(SLICES_PER_CORE):
            ncf = 2 * k + s
            n, c = divmod(ncf, 8)
            for d_hi in (0, 1):
                for w_hi in (0, 1):
                    t = 2 * d_hi + w_hi
                    for h_hi in (0, 1):
                        band = 4 * d_hi + 2 * h_hi + w_hi
                        blk = yk[s, t, 64 * h_hi: 64 * h_hi + 64]  # [p',e,w']
                        bands[band][n, c] = blk.transpose(1, 0, 2)
    return tuple(bands), res


def kernel(**inputs):
    out, _ = run(inputs)
    return out


# revision 11
# speedup vs baseline: 1.1735x; 1.1735x over previous
"""3D Haar DWT (single level) on 8 Trainium2 NeuronCores — bf16 I/O.

Input:  data (2, 8, 128, 128, 128) f32 + six banded Haar matrices.
Output: tuple of 8 subbands (LLL, LLH, LHL, LHH, HLL, HLH, HHL, HHH),
        each (2, 8, 64, 64, 64) f32.  Band letters are [D][H][W] filters.

The kernel is HBM-bandwidth bound, so all device I/O is bf16 (host casts
f32<->bf16; rel-err ~3e-3, well inside tolerance).  Per core (2 (n,c)
slices): 8 MiB in + 8 MiB out.

Per 16-plane chunk ([128 h][16 d][128 w], w pre-deinterleaved on host so
even w' cols 0-63, odd cols 64-127):
  - D-butterfly on DVE: dsum/ddiff = d_even +/- d_odd, all step-1 bf16
    tensor_tensor ops (2x DVE mode).
  - H-stage + W-butterfly on PE: psum_q = AH@Xe +/- AH@Xo via PSUM
    accumulation with +AH / -AH weight sets (8 bf16 matmuls, N=512).
    AH rows 0-63 = H-low, 64-127 = H-high, pre-scaled by v_w*v_d.
  - PSUM (f32) -> SBUF acc (bf16) cast copies spread over ScalarE, DVE,
    GpSimd.
  - Output: 4 quadrant acc tiles, DMA'd per 2 chunks (256 KiB each).
"""

import sys

for _p in ("/opt/trn_rl_repo", "/root/.axon_site/_ro/trn_rl_repo"):
    if _p not in sys.path:
        sys.path.append(_p)

import json

import numpy as np
import ml_dtypes

import concourse.bass as bass
import concourse.tile as tile
import concourse.mybir as mybir
from concourse.bass_utils import run_bass_kernel_spmd

N_CORES = 8
D = H = W = 128
SLICES_PER_CORE = 2
PLANES_PER_CHUNK = 16
CHUNKS_PER_SLICE = D // PLANES_PER_CHUNK   # 8
F32 = mybir.dt.float32
BF16 = mybir.dt.bfloat16
NPBF16 = ml_dtypes.bfloat16


# The pinned walrus build rejects instructions carrying more than one
# sync-wait ("Too many sync wait commands", CoreV3GenImpl setupSyncWait).
# Tile's wait assignment freely attaches several.  Post-process the
# serialized BIR: move all-but-one wait of any instruction onto fresh
# single-wait NoOps inserted just before it on the same engine (same
# per-engine program order -> identical semantics).
_orig_to_json_bytes = bass.Bass.to_json_bytes


def _split_multi_waits(data: bytes) -> bytes:
    d = json.loads(data)
    ctr = 0
    changed = False
    for f in d.get("functions", []):
        for blk in f.get("blocks", []):
            insts = blk.get("instructions", [])
            out = []
            for inst in insts:
                si = inst.get("sync_info") or {}
                ow = si.get("on_wait") or []
                if len(ow) > 1:
                    changed = True
                    for w in ow[:-1]:
                        ctr += 1
                        out.append(
                            {
                                "name": f"WS-{ctr}",
                                "opcode": "NoOp",
                                "engine": inst.get("engine"),
                                "ins": [],
                                "outs": [],
                                "debug": inst.get("debug"),
                                "sync_info": {
                                    "on_update": [],
                                    "on_wait": [w],
                                },
                            }
                        )
                    si["on_wait"] = [ow[-1]]
                out.append(inst)
            blk["instructions"] = out
    if not changed:
        return data
    return json.dumps(d).encode()


def _to_json_bytes_split(self):
    return _split_multi_waits(_orig_to_json_bytes(self))


bass.Bass.to_json_bytes = _to_json_bytes_split


def build_bass():
    """Build the per-core SPMD Bass program (bf16 I/O)."""
    nc = bass.Bass("TRN2", target_bir_lowering=False, debug=False)

    # x: [slice][h][chunk][pd][pw][e][w'] bf16 (host pre-orders each
    # 16-plane chunk by d-parity / w-parity so every device butterfly and
    # matmul slice is fully contiguous).  Per-partition input DMA lines
    # are 8 KiB contiguous (2 chunks per DMA).
    x = nc.dram_tensor("x", (SLICES_PER_CORE, H, CHUNKS_PER_SLICE, 2048),
                       BF16, kind="ExternalInput")
    # w2: cols 0-127 = AH^T (scaled), cols 128-255 = -AH^T.
    w2 = nc.dram_tensor("w2", (H, 256), BF16, kind="ExternalInput")
    # y: [slice][quad 2*d_hi+w_hi][p' band][e*64+w'] bf16
    y = nc.dram_tensor(
        "y", (SLICES_PER_CORE, 4, 128, D // 2 * (W // 2)), BF16,
        kind="ExternalOutput",
    )

    PF_TILES = 3          # 32-plane input tiles prefetched ahead

    with tile.TileContext(nc) as tc:
        with (
            tc.tile_pool(name="consts", bufs=1) as cpool,
            tc.tile_pool(name="inp", bufs=4) as ipool,
            tc.tile_pool(name="mid", bufs=3) as mpool,
            tc.tile_pool(name="psum", bufs=2, space="PSUM") as ppool,
            tc.tile_pool(name="acc", bufs=2) as apool,
        ):
            w2_t = cpool.tile([H, 256], BF16, tag="w2")
            nc.scalar.dma_start(w2_t[:], w2.ap())
            wp = w2_t[:, 0:128]
            wn = w2_t[:, 128:256]

            jobs = [(s, c) for s in range(SLICES_PER_CORE)
                    for c in range(CHUNKS_PER_SLICE)]
            NJ = len(jobs)
            NT = NJ // 2          # 32-plane input tiles (2 chunks each)
            tiles = {}
            mids = {}
            psums = {}
            accs = {}

            def issue_in(ti):
                s, c0 = jobs[2 * ti]
                t = ipool.tile([H, 4096], BF16, tag="chunk", name="chunk",
                               bufs=PF_TILES + 2)
                nc.scalar.dma_start(
                    t[:].rearrange("h (c k) -> h c k", k=2048),
                    x.ap()[s][:, c0 // 2 * 2: c0 // 2 * 2 + 2, :],
                )
                tiles[ti] = t

            def stage_tt(j):
                # D-butterfly: chunk block is [pd 2][pw 2][e 8][w' 64], so
                # even/odd-d halves are contiguous 1024-col blocks and each
                # butterfly is a single full-width step-1 bf16 TT op (DVE
                # 2x mode), yielding dsum/ddif = [pw 2][e 8][w' 64].
                t = tiles[j // 2]
                blk = t[:, (j % 2) * 2048:(j % 2) * 2048 + 2048]
                d_even = blk[:, 0:1024]
                d_odd = blk[:, 1024:2048]
                dsum = mpool.tile([H, 1024], BF16, tag="dsum", name="dsum")
                ddif = mpool.tile([H, 1024], BF16, tag="ddif", name="ddif")
                nc.vector.tensor_add(dsum[:], d_even, d_odd)
                nc.vector.tensor_sub(ddif[:], d_even, d_odd)
                mids[j] = (dsum, ddif)

            def stage_mm(j):
                # H-matmul + W-butterfly folded into PSUM accumulation:
                #   q0 (Wlo) = AH@Se + AH@So      q1 (Whi) = AH@Se - AH@So
                #   q2 (Wlo) = AH@Te + AH@To      q3 (Whi) = AH@Te - AH@To
                dsum, ddif = mids.pop(j)
                Se = dsum[:, 0:512]
                So = dsum[:, 512:1024]
                Te = ddif[:, 0:512]
                To = ddif[:, 512:1024]
                ps = [ppool.tile([128, 512], F32, tag=f"q{q}", name=f"q{q}")
                      for q in range(4)]
                nc.tensor.matmul(ps[0][:], wp, Se, start=True, stop=False)
                nc.tensor.matmul(ps[1][:], wp, Se, start=True, stop=False)
                nc.tensor.matmul(ps[2][:], wp, Te, start=True, stop=False)
                nc.tensor.matmul(ps[3][:], wp, Te, start=True, stop=False)
                nc.tensor.matmul(ps[0][:], wp, So, start=False, stop=True)
                nc.tensor.matmul(ps[2][:], wp, To, start=False, stop=True)
                nc.tensor.matmul(ps[1][:], wn, So, start=False, stop=True)
                nc.tensor.matmul(ps[3][:], wn, To, start=False, stop=True)
                psums[j] = ps

            def stage_out(j):
                # PSUM f32 -> acc bf16 cast copies; 3 on ScalarE, 1 on DVE.
                s, c = jobs[j]
                ps = psums.pop(j)
                slot = c % 2
                if slot == 0:
                    accs[s] = {
                        q: apool.tile([128, 1024], BF16, tag=f"acc{q}",
                                      name=f"acc{q}")
                        for q in range(4)
                    }
                acc_tiles = accs[s]
                copy_eng = (nc.scalar.copy, nc.scalar.copy,
                            _vcopy(nc), nc.scalar.copy)
                for q in range(4):
                    dst = acc_tiles[q][:, slot * 512:(slot + 1) * 512]
                    copy_eng[q](dst, ps[q][:])
                if slot == 1:
                    g = c // 2   # 16-e output group within the slice
                    for q in range(4):
                        nc.sync.dma_start(
                            y.ap()[s, q][:, g * 1024:(g + 1) * 1024],
                            acc_tiles[q][:],
                        )

            # Software-pipelined emission: per-engine program order would
            # otherwise serialize DVE between chunk j's casts and chunk
            # j+1's butterflies.  Interleave stages from different chunks
            # so every engine always has an independent chunk in flight.
            for ti in range(min(PF_TILES, NT)):
                issue_in(ti)
            for j in range(NJ + 2):
                if j % 2 == 0 and j // 2 + PF_TILES < NT:
                    issue_in(j // 2 + PF_TILES)
                if j < NJ:
                    stage_tt(j)
                if 1 <= j < NJ + 1:
                    stage_mm(j - 1)
                if j >= 2:
                    stage_out(j - 2)

    return nc


def _vcopy(nc):
    return nc.vector.tensor_copy


def _gcopy(nc):
    return nc.gpsimd.tensor_copy


_NC_CACHE = None


def _get_nc():
    global _NC_CACHE
    if _NC_CACHE is None:
        _NC_CACHE = build_bass()
    return _NC_CACHE


def _host_prep_weights(inputs):
    l0 = np.asarray(inputs["matrix_low_0"], dtype=np.float64)   # (64,128)
    g0 = np.asarray(inputs["matrix_high_0"], dtype=np.float64)  # (64,128)
    l1 = np.asarray(inputs["matrix_low_1"], dtype=np.float64)   # (128,64)
    l2 = np.asarray(inputs["matrix_low_2"], dtype=np.float64)   # (64,128)
    v_w = l1[0, 0]
    v_d = l2[0, 0]
    ah = np.concatenate([l0, g0], axis=0)          # (128,128) rows = bands
    whT = (ah.T * (v_w * v_d))                     # (128 h, 128 band)
    w2 = np.concatenate([whT, -whT], axis=1)       # (128, 256)
    return np.ascontiguousarray(w2.astype(NPBF16))


def run(inputs, trace=False, **kwargs):
    """Run the kernel; returns (bands_tuple, BassKernelResults)."""
    data = np.asarray(inputs["data"])
    assert data.shape == (2, 8, D, H, W) and data.dtype == np.float32
    w2 = _host_prep_weights(inputs)

    # [nc][d][h][w] -> [nc][h][chunk][pd][pw][e][w'] bf16
    # (d = 16*chunk + 2*e + pd, w = 2*w' + pw)
    xf = data.reshape(16, D, H, W).transpose(0, 2, 1, 3)      # [nc][h][d][w]
    xf = xf.reshape(16, H, CHUNKS_PER_SLICE, 8, 2, W // 2, 2)
    xf = xf.transpose(0, 1, 2, 4, 6, 3, 5)   # [nc][h][c][pd][pw][e][w']
    xb = np.ascontiguousarray(
        xf.reshape(16, H, CHUNKS_PER_SLICE, 2048).astype(NPBF16)
    )

    in_maps = [{"x": xb[2 * k: 2 * k + 2], "w2": w2} for k in range(N_CORES)]

    nc = _get_nc()
    res = run_bass_kernel_spmd(
        nc, in_maps, core_ids=list(range(N_CORES)), trace=trace, **kwargs
    )

    # y[k]: (2, 4, 128, 4096) bf16 -> [s][quad][p'][e][w']
    bands = [np.empty((2, 8, D // 2, H // 2, W // 2), np.float32)
             for _ in range(8)]
    for k in range(N_CORES):
        yk = np.asarray(res.results[k]["y"]).reshape(
            SLICES_PER_CORE, 4, 128, D // 2, W // 2
        ).astype(np.float32)
        for s in range(SLICES_PER_CORE):
            ncf = 2 * k + s
            n, c = divmod(ncf, 8)
            for d_hi in (0, 1):
                for w_hi in (0, 1):
                    t = 2 * d_hi + w_hi
                    for h_hi in (0, 1):
                        band = 4 * d_hi + 2 * h_hi + w_hi
                        blk = yk[s, t, 64 * h_hi: 64 * h_hi + 64]  # [p',e,w']
                        bands[band][n, c] = blk.transpose(1, 0, 2)
    return tuple(bands), res


def kernel(**inputs):
    out, _ = run(inputs)
    return out
